# revision 28
# baseline (speedup 1.0000x reference)
"""GQA attention kernel for 8 Trainium2 NeuronCores.

Sharding: tensor-parallel over kv-head groups x data-parallel over batch.
Core c handles batch b = c//4 and kv-head group g = c%4 (query heads
4g..4g+3) for ALL 2048 query positions of its batch. Wq/Wk/Wv are split
column-wise by head group, Wo row-wise; each core emits a partial output
projection and the host sums the 4 partials per batch (the "all-reduce
after output projection" of classic TP, done on the host). This removes
the K/V-projection duplication that pure sequence-parallel pays.

All activations are kept feature-major on-chip; the host pre-transposes
AND pre-packs every streamed tensor into its exact SBUF slab layout, so
each DMA line is >=4KB-contiguous (full DMA bandwidth) and the kernel
contains NO PE transposes. All matmul operands are fp16 (1 cycle/row
streaming at N=512 -> ~216ns/matmul, the PE floor); PSUM accumulation
stays fp32. The host un-transposes + reduces the output. Rel err ~5e-3.

Pipeline (emission order interleaves DMA-hungry Kproj blocks between
Qproj/Vproj blocks so the DMA engines never starve the PE):

  Q0 K0 Q1 K1 V0 Q2 K2 V1 Q3 K3 V2 V3   (projections, PSUM-quad blocks)
  for s4 (query 512-block): 4 heads of attention, then Oproj(s4)

  attention per (head, s4), per PAIR of skv chunks:
    scoresT pair -> one [128,1024] PSUM pair        (2 matmuls)
    PT = exp(scoresT*scale)                         (one ACT op per pair)
    rowsum += maskcol.T @ PT ; OT[h] += Vn.T @ PT   (PSUM acc, 2-pair skew)
  OT *= 1/rowsum: reciprocal on [1,512] (DVE) broadcast across partitions
  on the idle GpSimd engine -- no PE work; emitted lazily one iteration
  later so the PE never stalls on the epilogue.

The mask is applied by zeroing rows of Vn and using the mask itself as
the rowsum stationary vector (exp(-1e9)=0 equivalence), so the exp needs
no per-chunk bias and pairs of chunks share one ACT instruction.
"""

import os
import sys

sys.path.insert(0, "/opt/trn_rl_repo")
if os.environ.get("JAX_PLATFORMS") == "cpu":
    del os.environ["JAX_PLATFORMS"]
os.environ.setdefault("MYCRO_LOCAL_CACHE", "1")

from contextlib import ExitStack

import numpy as np

import concourse.bass as bass
import concourse.bacc as bacc
import concourse.mybir as mybir
import concourse.tile as tile

P = 128
E = 2048          # embed dim
S = 2048          # sequence length (queries and kv)
GQ = 512          # per-group query-projection width (4 heads * 128)
GK = 128          # per-group kv width (1 kv head)
nE = E // P       # 16
nKV = S // P      # 16
SC = 1.0 / float(128.0) ** 0.5
B = 2
N_CORES = 8

F32 = mybir.dt.float32
F16 = mybir.dt.float16
AF = mybir.ActivationFunctionType


def build_nc():
    nc = bacc.Bacc(target_bir_lowering=False)

    # activations prepacked on the host as [s4][p][e][512] slabs
    qt_d = nc.dram_tensor("qt", [4, P, nE, 512], F16, kind="ExternalInput")
    kt_d = nc.dram_tensor("kt", [4, P, nE, 512], F16, kind="ExternalInput")
    vt_d = nc.dram_tensor("vt", [4, P, nE, 512], F16, kind="ExternalInput")
    m_d = nc.dram_tensor("m", [S], F32, kind="ExternalInput")
    # weights prepacked as [p][e][cols]
    wq_d = nc.dram_tensor("wq", [P, nE, GQ], F16, kind="ExternalInput")
    wk_d = nc.dram_tensor("wk", [P, nE, GK], F16, kind="ExternalInput")
    wv_d = nc.dram_tensor("wv", [P, nE, GK], F16, kind="ExternalInput")
    wo_d = nc.dram_tensor("wo", [P, 4, E], F16, kind="ExternalInput")
    yt_d = nc.dram_tensor("yt", [E, S], F16, kind="ExternalOutput")

    with ExitStack() as ctx:
        tc = ctx.enter_context(tile.TileContext(nc))
        consts = ctx.enter_context(tc.tile_pool(name="consts", bufs=1))
        wqres = ctx.enter_context(tc.tile_pool(name="wqres", bufs=1))
        wkres = ctx.enter_context(tc.tile_pool(name="wkres", bufs=1))
        wvres = ctx.enter_context(tc.tile_pool(name="wvres", bufs=1))
        wores = ctx.enter_context(tc.tile_pool(name="wores", bufs=1))
        qslab = ctx.enter_context(tc.tile_pool(name="qslab", bufs=2))
        kvslab = ctx.enter_context(tc.tile_pool(name="kvslab", bufs=3))
        bigq = ctx.enter_context(tc.tile_pool(name="bigq", bufs=1))
        bigk = ctx.enter_context(tc.tile_pool(name="bigk", bufs=1))
        bigv = ctx.enter_context(tc.tile_pool(name="bigv", bufs=1))
        bigo = ctx.enter_context(tc.tile_pool(name="bigo", bufs=1))
        ptp = ctx.enter_context(tc.tile_pool(name="ptp", bufs=2))
        small = ctx.enter_context(tc.tile_pool(name="small", bufs=2))
        psmm = ctx.enter_context(tc.tile_pool(name="psmm", bufs=3, space="PSUM"))
        psra = ctx.enter_context(tc.tile_pool(name="psra", bufs=2, space="PSUM"))
        ystg = ctx.enter_context(tc.tile_pool(name="ystg", bufs=4))

        # ---- constants ----
        mask_sb = consts.tile([P, nKV], F32, tag="msk")
        nc.sync.dma_start(out=mask_sb, in_=m_d.rearrange("(a b) -> b a", b=P))
        mcol = consts.tile([P, nKV], F16, tag="mcol")
        nc.vector.tensor_copy(mcol, mask_sb)

        # ---- resident weights; wq interleaved with the first q slab so
        # ---- the PE starts streaming within ~3us of kernel start
        wqr = wqres.tile([P, nE, GQ], F16, tag="wq")
        QT = bigq.tile([P, 4, S], F16, tag="qt")
        qsl0 = qslab.tile([P, nE, 512], F16, tag="q", name="qsl0")
        # 1-e granularity for the first tiles so the PE starts ~1us in,
        # then coarser chunks for efficiency
        for sl in ([slice(e, e + 1) for e in range(4)]
                   + [slice(4 + 4 * q, 8 + 4 * q) for q in range(3)]):
            nc.sync.dma_start(out=wqr[:, sl, :], in_=wq_d[:, sl, :])
            nc.sync.dma_start(out=qsl0[:, sl, :], in_=qt_d[0][:, sl, :])
        # PE warmup: junk matmuls on the first wq slice while the rest of
        # the startup DMAs land, so the HAM clock gate reaches 2.4GHz before
        # the real stream begins. Two psra allocations keep rotation parity.
        ps_w0 = psra.tile([P, 512], F32, tag="ra", name="ps_w0")
        ps_w1 = psra.tile([P, 512], F32, tag="ra", name="ps_w1")
        for _w in range(7):
            nc.tensor.matmul(ps_w0, wqr[:, 0, 0:128], wqr[:, 0, :],
                             start=True, stop=True, skip_group_check=True)
            nc.tensor.matmul(ps_w1, wqr[:, 0, 0:128], wqr[:, 0, :],
                             start=True, stop=True, skip_group_check=True)
        wkr = wkres.tile([P, nE, GK], F16, tag="wk")
        nc.sync.dma_start(out=wkr, in_=wk_d[:, :, :])
        wvr = wvres.tile([P, nE, GK], F16, tag="wv")
        nc.sync.dma_start(out=wvr, in_=wv_d[:, :, :])

        def load_slab(pool, src_d, s4, tag):
            sl = pool.tile([P, nE, 512], F16, tag=tag, name=f"slab_{tag}")
            for q in range(4):
                nc.sync.dma_start(
                    out=sl[:, q * 4:(q + 1) * 4, :],
                    in_=src_d[s4][:, q * 4:(q + 1) * 4, :],
                )
            return sl

        def quad_psum():
            prs = [psmm.tile([P, 2, 512], F32, tag="mm", name=f"pr{_i}")
                   for _i in range(2)]
            return [prs[_j // 2][:, _j % 2, :] for _j in range(4)]

        # ---- projection blocks ----
        KT = bigk.tile([P, S], F16, tag="kt")
        Vn = bigv.tile([P, nKV, GK], F16, tag="vn")

        def q_block(s4, qsl):
            pss = quad_psum()
            for e in range(nE):
                for hc in range(4):
                    nc.tensor.matmul(
                        pss[hc], wqr[:, e, hc * 128:(hc + 1) * 128], qsl[:, e, :],
                        start=(e == 0), stop=(e == nE - 1), skip_group_check=True,
                    )
            for hc in range(4):
                nc.vector.tensor_copy(QT[:, hc, s4 * 512:(s4 + 1) * 512], pss[hc])

        def k_block(cs):
            ksl = load_slab(kvslab, kt_d, cs, "kv")
            pr = psmm.tile([P, 2, 512], F32, tag="mm", name="prk")
            for e in range(nE):
                nc.tensor.matmul(
                    pr[:, 0, :], wkr[:, e, :], ksl[:, e, :],
                    start=(e == 0), stop=(e == nE - 1), skip_group_check=True,
                )
            nc.vector.tensor_copy(KT[:, cs * 512:(cs + 1) * 512], pr[:, 0, :])

        def v_block(mq, vsl=None):
            if vsl is None:
                vsl = load_slab(kvslab, vt_d, mq, "kv")
            pss = quad_psum()
            for e in range(nE):
                for j in range(4):
                    nc.tensor.matmul(
                        pss[j][:, 0:GK], vsl[:, e, j * 128:(j + 1) * 128],
                        wvr[:, e, :],
                        start=(e == 0), stop=(e == nE - 1), skip_group_check=True,
                    )
            for j in range(4):
                c = mq * 4 + j
                # rows of V for masked skv positions are zeroed here
                nc.vector.tensor_scalar_mul(
                    Vn[:, c, :], pss[j][:, 0:GK], mask_sb[:, c:c + 1]
                )

        # interleave: K blocks are DMA-hungry (2MB per 3.5us of PE work),
        # so they sit between Q/V blocks and prefetch during them.
        q_block(0, qsl0)
        k_block(0)
        q_block(1, load_slab(qslab, qt_d, 1, "q"))
        k_block(1)
        v_block(0)
        q_block(2, load_slab(qslab, qt_d, 2, "q"))
        k_block(2)
        v_block(1)
        q_block(3, load_slab(qslab, qt_d, 3, "q"))
        k_block(3)
        vsl2 = load_slab(kvslab, vt_d, 2, "kv")
        vsl3 = load_slab(kvslab, vt_d, 3, "kv")
        v_block(2, vsl2)
        v_block(3, vsl3)

        # resident Wo: needed in Oproj, DMA hides under early attention
        wor = wores.tile([P, 4, E], F16, tag="wo")
        nc.sync.dma_start(out=wor, in_=wo_d[:, :, :])

        # ---- attention + output projection, query-block-major ----
        OT = bigo.tile([P, 4, S], F16, tag="ot")
        pending = None        # lazy epilogue: (ps_rs, ps_av, h, s4)
        pending_tails = []    # previous iteration's last two rs/av pairs

        def flush_tail():
            if pending_tails:
                pending_tails.pop(0)()

        def flush_epilogue():
            nonlocal pending
            if pending is None:
                return
            ps_rs, av_sb, h, s4 = pending
            pending = None
            # reciprocal of the [1,512] denominator, broadcast to all 128
            # partitions on the (otherwise idle) GpSimd engine -- no PE work
            recip_sm = small.tile([1, 512], F32, tag="recip_sm")
            nc.vector.reciprocal_approx_fast(out=recip_sm, in_=ps_rs)
            recip_bc = small.tile([P, 512], F32, tag="recip_bc")
            nc.gpsimd.partition_broadcast(recip_bc, recip_sm, channels=P)
            nc.vector.tensor_mul(
                OT[:, h, s4 * 512:(s4 + 1) * 512], av_sb, recip_bc
            )

        for s4 in range(4):
            for h in range(4):
                qs = QT[:, h, s4 * 512:(s4 + 1) * 512]
                ps_rs = psra.tile([1, 512], F32, tag="ra")
                ps_av = psra.tile([P, 512], F32, tag="ra")
                PTh = [None, None]

                def rs_av(c, PTh=PTh, ps_rs=ps_rs, ps_av=ps_av):
                    pt_c = PTh[c // 8][:, c % 8, :]
                    nc.tensor.matmul(
                        ps_rs, mcol[:, c:c + 1], pt_c,
                        start=(c == 0), stop=(c == nKV - 1),
                        skip_group_check=True,
                    )
                    nc.tensor.matmul(
                        ps_av, Vn[:, c, :], pt_c,
                        start=(c == 0), stop=(c == nKV - 1),
                        skip_group_check=True,
                    )

                for p in range(nKV // 2):  # pairs of skv chunks
                    c0 = 2 * p
                    if c0 % 8 == 0:
                        PTh[c0 // 8] = ptp.tile([P, 8, 512], F16, tag="pt",
                                                name="PTh")
                    ps_s = psmm.tile([P, 2, 512], F32, tag="mm", name="ps_s")
                    for i in range(2):
                        nc.tensor.matmul(
                            ps_s[:, i, :],
                            KT[:, (c0 + i) * 128:(c0 + i + 1) * 128],
                            qs, start=True, stop=True,
                        )
                    if p in (0, 1):
                        # previous iteration's last rs/av pairs land under
                        # this iteration's first exp latencies
                        flush_tail()
                    if p == 2:
                        flush_epilogue()
                    nc.scalar.activation(
                        PTh[c0 // 8][:, c0 % 8:c0 % 8 + 2, :], ps_s, AF.Exp,
                        scale=SC,
                    )
                    if p >= 2:
                        # two-pair skew: the exp feeding these rs/av matmuls
                        # finished ~2.6us ago, so sem jitter never stalls PE
                        rs_av(c0 - 4)
                        rs_av(c0 - 3)
                av_sb = small.tile([P, 512], F32, tag="av_sb")

                def tail2(rs_av=rs_av, ps_av=ps_av, av_sb=av_sb):
                    rs_av(nKV - 2)
                    rs_av(nKV - 1)
                    # evacuate the AV accumulator to SBUF right away: the
                    # next iteration's first av matmul reuses this PSUM bank
                    # and must not wait for the recip->broadcast->mul chain
                    nc.vector.tensor_copy(av_sb, ps_av)

                pending_tails.extend([
                    lambda rs_av=rs_av: (rs_av(nKV - 4), rs_av(nKV - 3)),
                    tail2,
                ])
                pending = (ps_rs, av_sb, h, s4)

            # head 3 of this s4 must fully land before Oproj(s4)
            flush_tail()
            flush_tail()
            flush_epilogue()

            for q4 in range(4):
                pss = quad_psum()
                for o in range(4):
                    for j in range(4):
                        nc.tensor.matmul(
                            pss[j],
                            wor[:, o, (q4 * 4 + j) * 128:(q4 * 4 + j + 1) * 128],
                            OT[:, o, s4 * 512:(s4 + 1) * 512],
                            start=(o == 0), stop=(o == 3), skip_group_check=True,
                        )
                for j in range(4):
                    ys = ystg.tile([P, 512], F16, tag="y")
                    nc.vector.tensor_copy(ys, pss[j])
                    nc.sync.dma_start(
                        out=yt_d[(q4 * 4 + j) * 128:(q4 * 4 + j + 1) * 128,
                                 s4 * 512:(s4 + 1) * 512],
                        in_=ys,
                    )

    nc.compile()
    return nc


_nc = None


def _get_nc():
    global _nc
    if _nc is None:
        _nc = build_nc()
    return _nc


def _pack_act(x):
    # [S, E] fp32 -> transposed slab layout [4(s4), P, nE, 512] fp16
    xt = np.asarray(x, np.float32).T.astype(np.float16)        # [E, S]
    return np.ascontiguousarray(
        xt.reshape(nE, P, 4, 512).transpose(2, 1, 0, 3)
    )


def _pack_w(w, ncols):
    # [E, ncols] fp32 -> [P, nE, ncols] fp16
    wh = np.asarray(w, np.float32).astype(np.float16)
    return np.ascontiguousarray(wh.reshape(nE, P, ncols).transpose(1, 0, 2))


def _make_in_maps(query, key, value, mask, Wq, Wk, Wv, Wo):
    qts = [_pack_act(query[b]) for b in range(B)]
    kts = [_pack_act(key[b]) for b in range(B)]
    vts = [_pack_act(value[b]) for b in range(B)]
    ms = [np.ascontiguousarray(mask[b], dtype=np.float32) for b in range(B)]
    wq_f, wk_f, wv_f, wo_f = (np.asarray(w, np.float32) for w in (Wq, Wk, Wv, Wo))
    in_maps = []
    for c in range(N_CORES):
        b, g = c // 4, c % 4
        wo_slice = wo_f[g * GQ:(g + 1) * GQ, :].astype(np.float16)  # [512, E]
        in_maps.append({
            "qt": qts[b],
            "kt": kts[b],
            "vt": vts[b],
            "m": ms[b],
            "wq": _pack_w(wq_f[:, g * GQ:(g + 1) * GQ], GQ),
            "wk": _pack_w(wk_f[:, g * GK:(g + 1) * GK], GK),
            "wv": _pack_w(wv_f[:, g * GK:(g + 1) * GK], GK),
            "wo": np.ascontiguousarray(
                wo_slice.reshape(4, P, E).transpose(1, 0, 2)
            ),
        })
    return in_maps


def run(query, key, value, mask, Wq, Wk, Wv, Wo, trace=False, trace_kwargs=None):
    from concourse.bass_utils import run_bass_kernel_spmd

    nc = _get_nc()
    in_maps = _make_in_maps(query, key, value, mask, Wq, Wk, Wv, Wo)
    res = run_bass_kernel_spmd(
        nc, in_maps, list(range(N_CORES)), trace=trace, **(trace_kwargs or {})
    )
    out = np.empty((B, S, E), np.float32)
    for b in range(B):
        acc = np.zeros((E, S), np.float32)
        for g in range(4):
            acc += res.results[b * 4 + g]["yt"].astype(np.float32)
        out[b] = acc.T
    return out, res


def kernel(query, key, value, mask, Wq, Wk, Wv, Wo):
    out, _ = run(query, key, value, mask, Wq, Wk, Wv, Wo, trace=False)
    return out


# revision 29
# speedup vs baseline: 1.0130x; 1.0130x over previous
"""GQA attention kernel for 8 Trainium2 NeuronCores.

Sharding: tensor-parallel over kv-head groups x data-parallel over batch.
Core c handles batch b = c//4 and kv-head group g = c%4 (query heads
4g..4g+3) for ALL 2048 query positions of its batch. Wq/Wk/Wv are split
column-wise by head group, Wo row-wise; each core emits a partial output
projection and the host sums the 4 partials per batch (the "all-reduce
after output projection" of classic TP, done on the host). This removes
the K/V-projection duplication that pure sequence-parallel pays.

All activations are kept feature-major on-chip; the host pre-transposes
AND pre-packs every streamed tensor into its exact SBUF slab layout, so
each DMA line is >=4KB-contiguous (full DMA bandwidth) and the kernel
contains NO PE transposes. All matmul operands are fp16 (1 cycle/row
streaming at N=512 -> ~216ns/matmul, the PE floor); PSUM accumulation
stays fp32. The host un-transposes + reduces the output. Rel err ~5e-3.

Pipeline (emission order interleaves DMA-hungry Kproj blocks between
Qproj/Vproj blocks so the DMA engines never starve the PE):

  Q0 K0 Q1 K1 V0 Q2 K2 V1 Q3 K3 V2 V3   (projections, PSUM-quad blocks)
  for s4 (query 512-block): 4 heads of attention, then Oproj(s4)

  attention per (head, s4), per PAIR of skv chunks:
    scoresT pair -> one [128,1024] PSUM pair        (2 matmuls)
    PT = exp(scoresT*scale)                         (one ACT op per pair)
    rowsum += maskcol.T @ PT ; OT[h] += Vn.T @ PT   (PSUM acc, 2-pair skew)
  OT *= 1/rowsum: reciprocal on [1,512] (DVE) broadcast across partitions
  on the idle GpSimd engine -- no PE work; emitted lazily one iteration
  later so the PE never stalls on the epilogue.

The mask is applied by zeroing rows of Vn and using the mask itself as
the rowsum stationary vector (exp(-1e9)=0 equivalence), so the exp needs
no per-chunk bias and pairs of chunks share one ACT instruction.
"""

import os
import sys

sys.path.insert(0, "/opt/trn_rl_repo")
if os.environ.get("JAX_PLATFORMS") == "cpu":
    del os.environ["JAX_PLATFORMS"]
os.environ.setdefault("MYCRO_LOCAL_CACHE", "1")

from contextlib import ExitStack

import numpy as np

import concourse.bass as bass
import concourse.bacc as bacc
import concourse.mybir as mybir
import concourse.tile as tile

P = 128
E = 2048          # embed dim
S = 2048          # sequence length (queries and kv)
GQ = 512          # per-group query-projection width (4 heads * 128)
GK = 128          # per-group kv width (1 kv head)
nE = E // P       # 16
nKV = S // P      # 16
SC = 1.0 / float(128.0) ** 0.5
B = 2
N_CORES = 8

F32 = mybir.dt.float32
F16 = mybir.dt.float16
AF = mybir.ActivationFunctionType


def build_nc():
    nc = bacc.Bacc(target_bir_lowering=False)

    # activations prepacked on the host as [s4][p][e][512] slabs
    qt_d = nc.dram_tensor("qt", [4, P, nE, 512], F16, kind="ExternalInput")
    kt_d = nc.dram_tensor("kt", [4, P, nE, 512], F16, kind="ExternalInput")
    vt_d = nc.dram_tensor("vt", [4, P, nE, 512], F16, kind="ExternalInput")
    m_d = nc.dram_tensor("m", [S], F32, kind="ExternalInput")
    # weights prepacked as [p][e][cols]
    wq_d = nc.dram_tensor("wq", [P, nE, GQ], F16, kind="ExternalInput")
    wk_d = nc.dram_tensor("wk", [P, nE, GK], F16, kind="ExternalInput")
    wv_d = nc.dram_tensor("wv", [P, nE, GK], F16, kind="ExternalInput")
    wo_d = nc.dram_tensor("wo", [P, 4, E], F16, kind="ExternalInput")
    yt_d = nc.dram_tensor("yt", [E, S], F16, kind="ExternalOutput")

    with ExitStack() as ctx:
        tc = ctx.enter_context(tile.TileContext(nc))
        consts = ctx.enter_context(tc.tile_pool(name="consts", bufs=1))
        wqres = ctx.enter_context(tc.tile_pool(name="wqres", bufs=1))
        wkres = ctx.enter_context(tc.tile_pool(name="wkres", bufs=1))
        wvres = ctx.enter_context(tc.tile_pool(name="wvres", bufs=1))
        wores = ctx.enter_context(tc.tile_pool(name="wores", bufs=1))
        qslab = ctx.enter_context(tc.tile_pool(name="qslab", bufs=2))
        kvslab = ctx.enter_context(tc.tile_pool(name="kvslab", bufs=3))
        bigq = ctx.enter_context(tc.tile_pool(name="bigq", bufs=1))
        bigk = ctx.enter_context(tc.tile_pool(name="bigk", bufs=1))
        bigv = ctx.enter_context(tc.tile_pool(name="bigv", bufs=1))
        bigo = ctx.enter_context(tc.tile_pool(name="bigo", bufs=1))
        ptp = ctx.enter_context(tc.tile_pool(name="ptp", bufs=2))
        small = ctx.enter_context(tc.tile_pool(name="small", bufs=2))
        psmm = ctx.enter_context(tc.tile_pool(name="psmm", bufs=3, space="PSUM"))
        psra = ctx.enter_context(tc.tile_pool(name="psra", bufs=2, space="PSUM"))
        ystg = ctx.enter_context(tc.tile_pool(name="ystg", bufs=4))

        # ---- constants ----
        mask_sb = consts.tile([P, nKV], F32, tag="msk")
        nc.sync.dma_start(out=mask_sb, in_=m_d.rearrange("(a b) -> b a", b=P))
        mcol = consts.tile([P, nKV], F16, tag="mcol")
        nc.vector.tensor_copy(mcol, mask_sb)

        # ---- resident weights; wq interleaved with the first q slab so
        # ---- the PE starts streaming within ~3us of kernel start
        wqr = wqres.tile([P, nE, GQ], F16, tag="wq")
        QT = bigq.tile([P, 4, S], F16, tag="qt")
        qsl0 = qslab.tile([P, nE, 512], F16, tag="q", name="qsl0")
        # 1-e granularity for the first tiles so the PE starts ~1us in,
        # then coarser chunks for efficiency
        for sl in ([slice(e, e + 1) for e in range(4)]
                   + [slice(4 + 4 * q, 8 + 4 * q) for q in range(3)]):
            nc.sync.dma_start(out=wqr[:, sl, :], in_=wq_d[:, sl, :])
            nc.sync.dma_start(out=qsl0[:, sl, :], in_=qt_d[0][:, sl, :])
        wkr = wkres.tile([P, nE, GK], F16, tag="wk")
        nc.sync.dma_start(out=wkr, in_=wk_d[:, :, :])
        wvr = wvres.tile([P, nE, GK], F16, tag="wv")
        nc.sync.dma_start(out=wvr, in_=wv_d[:, :, :])

        def load_slab(pool, src_d, s4, tag):
            sl = pool.tile([P, nE, 512], F16, tag=tag, name=f"slab_{tag}")
            for q in range(4):
                nc.sync.dma_start(
                    out=sl[:, q * 4:(q + 1) * 4, :],
                    in_=src_d[s4][:, q * 4:(q + 1) * 4, :],
                )
            return sl

        def quad_psum():
            prs = [psmm.tile([P, 2, 512], F32, tag="mm", name=f"pr{_i}")
                   for _i in range(2)]
            return [prs[_j // 2][:, _j % 2, :] for _j in range(4)]

        # ---- projection blocks ----
        KT = bigk.tile([P, S], F16, tag="kt")
        Vn = bigv.tile([P, nKV, GK], F16, tag="vn")

        def q_block(s4, qsl):
            pss = quad_psum()
            for e in range(nE):
                for hc in range(4):
                    nc.tensor.matmul(
                        pss[hc], wqr[:, e, hc * 128:(hc + 1) * 128], qsl[:, e, :],
                        start=(e == 0), stop=(e == nE - 1), skip_group_check=True,
                    )
            for hc in range(4):
                nc.vector.tensor_copy(QT[:, hc, s4 * 512:(s4 + 1) * 512], pss[hc])

        def k_block(cs):
            ksl = load_slab(kvslab, kt_d, cs, "kv")
            pr = psmm.tile([P, 2, 512], F32, tag="mm", name="prk")
            for e in range(nE):
                nc.tensor.matmul(
                    pr[:, 0, :], wkr[:, e, :], ksl[:, e, :],
                    start=(e == 0), stop=(e == nE - 1), skip_group_check=True,
                )
            nc.vector.tensor_copy(KT[:, cs * 512:(cs + 1) * 512], pr[:, 0, :])

        def v_block(mq):
            vsl = load_slab(kvslab, vt_d, mq, "kv")
            pss = quad_psum()
            for e in range(nE):
                for j in range(4):
                    nc.tensor.matmul(
                        pss[j][:, 0:GK], vsl[:, e, j * 128:(j + 1) * 128],
                        wvr[:, e, :],
                        start=(e == 0), stop=(e == nE - 1), skip_group_check=True,
                    )
            for j in range(4):
                c = mq * 4 + j
                # rows of V for masked skv positions are zeroed here
                nc.vector.tensor_scalar_mul(
                    Vn[:, c, :], pss[j][:, 0:GK], mask_sb[:, c:c + 1]
                )

        # interleave: K blocks are DMA-hungry (2MB per 3.5us of PE work),
        # so they sit between Q/V blocks and prefetch during them.
        q_block(0, qsl0)
        k_block(0)
        q_block(1, load_slab(qslab, qt_d, 1, "q"))
        k_block(1)
        v_block(0)
        q_block(2, load_slab(qslab, qt_d, 2, "q"))
        k_block(2)
        v_block(1)
        q_block(3, load_slab(qslab, qt_d, 3, "q"))
        k_block(3)
        v_block(2)
        v_block(3)

        # resident Wo: needed in Oproj, DMA hides under early attention
        wor = wores.tile([P, 4, E], F16, tag="wo")
        nc.sync.dma_start(out=wor, in_=wo_d[:, :, :])

        # ---- attention + output projection, query-block-major ----
        OT = bigo.tile([P, 4, S], F16, tag="ot")
        pending = None        # lazy epilogue: (ps_rs, ps_av, h, s4)
        pending_tails = []    # previous iteration's last two rs/av pairs

        def flush_tail():
            if pending_tails:
                pending_tails.pop(0)()

        def flush_epilogue():
            nonlocal pending
            if pending is None:
                return
            ps_rs, av_sb, h, s4 = pending
            pending = None
            # reciprocal of the [1,512] denominator, broadcast to all 128
            # partitions on the (otherwise idle) GpSimd engine -- no PE work
            recip_sm = small.tile([1, 512], F32, tag="recip_sm")
            nc.vector.reciprocal_approx_fast(out=recip_sm, in_=ps_rs)
            recip_bc = small.tile([P, 512], F32, tag="recip_bc")
            nc.gpsimd.partition_broadcast(recip_bc, recip_sm, channels=P)
            nc.vector.tensor_mul(
                OT[:, h, s4 * 512:(s4 + 1) * 512], av_sb, recip_bc
            )

        for s4 in range(4):
            for h in range(4):
                qs = QT[:, h, s4 * 512:(s4 + 1) * 512]
                ps_rs = psra.tile([1, 512], F32, tag="ra")
                ps_av = psra.tile([P, 512], F32, tag="ra")
                PTh = [None, None]

                def rs_av(c, PTh=PTh, ps_rs=ps_rs, ps_av=ps_av):
                    pt_c = PTh[c // 8][:, c % 8, :]
                    nc.tensor.matmul(
                        ps_rs, mcol[:, c:c + 1], pt_c,
                        start=(c == 0), stop=(c == nKV - 1),
                        skip_group_check=True,
                    )
                    nc.tensor.matmul(
                        ps_av, Vn[:, c, :], pt_c,
                        start=(c == 0), stop=(c == nKV - 1),
                        skip_group_check=True,
                    )

                for p in range(nKV // 2):  # pairs of skv chunks
                    c0 = 2 * p
                    if c0 % 8 == 0:
                        PTh[c0 // 8] = ptp.tile([P, 8, 512], F16, tag="pt",
                                                name="PTh")
                    ps_s = psmm.tile([P, 2, 512], F32, tag="mm", name="ps_s")
                    for i in range(2):
                        nc.tensor.matmul(
                            ps_s[:, i, :],
                            KT[:, (c0 + i) * 128:(c0 + i + 1) * 128],
                            qs, start=True, stop=True,
                        )
                    if p in (0, 1):
                        # previous iteration's last rs/av pairs land under
                        # this iteration's first exp latencies
                        flush_tail()
                    if p == 2:
                        flush_epilogue()
                    nc.scalar.activation(
                        PTh[c0 // 8][:, c0 % 8:c0 % 8 + 2, :], ps_s, AF.Exp,
                        scale=SC,
                    )
                    if p >= 2:
                        # two-pair skew: the exp feeding these rs/av matmuls
                        # finished ~2.6us ago, so sem jitter never stalls PE
                        rs_av(c0 - 4)
                        rs_av(c0 - 3)
                av_sb = small.tile([P, 512], F32, tag="av_sb")

                def tail2(rs_av=rs_av, ps_av=ps_av, av_sb=av_sb):
                    rs_av(nKV - 2)
                    rs_av(nKV - 1)
                    # evacuate the AV accumulator to SBUF right away: the
                    # next iteration's first av matmul reuses this PSUM bank
                    # and must not wait for the recip->broadcast->mul chain
                    nc.vector.tensor_copy(av_sb, ps_av)

                pending_tails.extend([
                    lambda rs_av=rs_av: (rs_av(nKV - 4), rs_av(nKV - 3)),
                    tail2,
                ])
                pending = (ps_rs, av_sb, h, s4)

            # head 3 of this s4 must fully land before Oproj(s4)
            flush_tail()
            flush_tail()
            flush_epilogue()

            for q4 in range(4):
                pss = quad_psum()
                for o in range(4):
                    for j in range(4):
                        nc.tensor.matmul(
                            pss[j],
                            wor[:, o, (q4 * 4 + j) * 128:(q4 * 4 + j + 1) * 128],
                            OT[:, o, s4 * 512:(s4 + 1) * 512],
                            start=(o == 0), stop=(o == 3), skip_group_check=True,
                        )
                for j in range(4):
                    ys = ystg.tile([P, 512], F16, tag="y")
                    nc.vector.tensor_copy(ys, pss[j])
                    nc.sync.dma_start(
                        out=yt_d[(q4 * 4 + j) * 128:(q4 * 4 + j + 1) * 128,
                                 s4 * 512:(s4 + 1) * 512],
                        in_=ys,
                    )

    nc.compile()
    return nc


_nc = None


def _get_nc():
    global _nc
    if _nc is None:
        _nc = build_nc()
    return _nc


def _pack_act(x):
    # [S, E] fp32 -> transposed slab layout [4(s4), P, nE, 512] fp16
    xt = np.asarray(x, np.float32).T.astype(np.float16)        # [E, S]
    return np.ascontiguousarray(
        xt.reshape(nE, P, 4, 512).transpose(2, 1, 0, 3)
    )


def _pack_w(w, ncols):
    # [E, ncols] fp32 -> [P, nE, ncols] fp16
    wh = np.asarray(w, np.float32).astype(np.float16)
    return np.ascontiguousarray(wh.reshape(nE, P, ncols).transpose(1, 0, 2))


def _make_in_maps(query, key, value, mask, Wq, Wk, Wv, Wo):
    qts = [_pack_act(query[b]) for b in range(B)]
    kts = [_pack_act(key[b]) for b in range(B)]
    vts = [_pack_act(value[b]) for b in range(B)]
    ms = [np.ascontiguousarray(mask[b], dtype=np.float32) for b in range(B)]
    wq_f, wk_f, wv_f, wo_f = (np.asarray(w, np.float32) for w in (Wq, Wk, Wv, Wo))
    in_maps = []
    for c in range(N_CORES):
        b, g = c // 4, c % 4
        wo_slice = wo_f[g * GQ:(g + 1) * GQ, :].astype(np.float16)  # [512, E]
        in_maps.append({
            "qt": qts[b],
            "kt": kts[b],
            "vt": vts[b],
            "m": ms[b],
            "wq": _pack_w(wq_f[:, g * GQ:(g + 1) * GQ], GQ),
            "wk": _pack_w(wk_f[:, g * GK:(g + 1) * GK], GK),
            "wv": _pack_w(wv_f[:, g * GK:(g + 1) * GK], GK),
            "wo": np.ascontiguousarray(
                wo_slice.reshape(4, P, E).transpose(1, 0, 2)
            ),
        })
    return in_maps


def run(query, key, value, mask, Wq, Wk, Wv, Wo, trace=False, trace_kwargs=None):
    from concourse.bass_utils import run_bass_kernel_spmd

    nc = _get_nc()
    in_maps = _make_in_maps(query, key, value, mask, Wq, Wk, Wv, Wo)
    res = run_bass_kernel_spmd(
        nc, in_maps, list(range(N_CORES)), trace=trace, **(trace_kwargs or {})
    )
    out = np.empty((B, S, E), np.float32)
    for b in range(B):
        acc = np.zeros((E, S), np.float32)
        for g in range(4):
            acc += res.results[b * 4 + g]["yt"].astype(np.float32)
        out[b] = acc.T
    return out, res


def kernel(query, key, value, mask, Wq, Wk, Wv, Wo):
    out, _ = run(query, key, value, mask, Wq, Wk, Wv, Wo, trace=False)
    return out
